# revision 41
# baseline (speedup 1.0000x reference)
"""Cross-attention kernel for Trainium2, 8-core SPMD.

Problem (all fp32):
  x [2, 2048, 1024]; wq/wk/wv/w_proj [1024, 1024]; b_proj [1024]
  q = x[:, :1024] @ wq.T   (16 heads x 64)
  k, v = x @ wk.T, x @ wv.T
  out = softmax(q k^T / 8) v  -> proj + bias  -> [2, 1024, 1024]

Sharding: 8 cores = 2 (batch) x 4 (head-groups of 4 heads). Each core
computes its batch's QKV for its 4 heads, full attention for those heads,
and a partial projection (its 256 contraction rows of w_proj). Host sums
the 4 bf16 partials per batch in fp32 and adds the bias.

Numerics: all DRAM inputs are bf16 (halves the input DMA, which bounds
how early the exp stream can start); q/k and the scores matmul stay in
fp32(r); exp(scores), v, attention output and the projection run in bf16
(validated 5.4e-3 max rel err vs the 2e-2 gate).

Layouts: x is kept feature-on-partition (xT [c, n]); q/k are produced
transposed (qT/kT [dh_pair, n]); v natural [n, d] with an appended
ones-column so attn@v also emits the softmax denominator. attn@v is
oriented [q, d] (queries on partitions): the output free size is 65
instead of 512, which halves its PE cost since PE time is
output-free-size * steps. The per-(head, q-tile) [128, 65] PSUM result is
normalized during evacuation (reciprocal of the denominator column +
per-partition tensor_scalar broadcast), PE-transposed into a packed
[2-heads x 64, q] tile via tile_position, and the projection then runs
with full 128-row stationary operands (half the naive cost).

Schedule: the ACT engine's exp stream (~66us: 64 x [128,1024] exps) is
the stage-B pacer; scores(h+1) and attnv(h) interleave per q-tile so ACT
never starves, and the projection pipelines per-q-tile inside head 3's
exp window. Stage A is paced by the 5MB x+wq+wk DMA stream (~19us);
scores(0)/exp(0) start immediately after the last x chunk, with the
remaining q/k/v work used as PE filler between them.
"""

import os
import numpy as np
import ml_dtypes

import concourse.bacc as bacc
import concourse.bass as bass
import concourse.tile as tile
import concourse.mybir as mybir
from concourse.bass_utils import run_bass_kernel_spmd

F32 = mybir.dt.float32
F32R = mybir.dt.float32r
BF16 = mybir.dt.bfloat16

C = 1024          # model dim
N = 2048          # kv tokens
NQ = 1024         # query tokens
HPC = 4           # heads per core
D = 64            # head dim
DH = HPC * D      # per-core slice of C (256)
SCALE = D ** -0.5
P = 128

_CACHE: dict = {}


def _build():
    nc = bacc.Bacc("TRN2", target_bir_lowering=False, debug=False, num_devices=8)

    xT = nc.dram_tensor("xT", [C, N], BF16, kind="ExternalInput").ap()
    wqT = nc.dram_tensor("wqT", [C, DH], BF16, kind="ExternalInput").ap()
    wkT = nc.dram_tensor("wkT", [C, DH], BF16, kind="ExternalInput").ap()
    wvT = nc.dram_tensor("wvT", [C, DH], BF16, kind="ExternalInput").ap()
    wpT = nc.dram_tensor("wpT", [DH, C], BF16, kind="ExternalInput").ap()
    ident = nc.dram_tensor("ident", [P, P], BF16, kind="ExternalInput").ap()
    out_a = nc.dram_tensor("out_a", [NQ, C], BF16, kind="ExternalOutput").ap()
    out_b = nc.dram_tensor("out_b", [NQ, C], BF16, kind="ExternalOutput").ap()

    with tile.TileContext(nc) as tc, \
            nc.allow_low_precision(reason="bf16/fp32r attention pipeline, validated 5.4e-3 rel err"):
        _emit(tc, xT, wqT, wkT, wvT, wpT, ident, out_a, out_b)

    nc.compile()
    return nc


def _emit(tc, xT, wqT, wkT, wvT, wpT, ident, out_a, out_b):
    nc = tc.nc
    mm = nc.tensor.matmul
    Exp = mybir.ActivationFunctionType.Exp

    from contextlib import ExitStack

    with ExitStack() as ctx:
        xp = ctx.enter_context(tc.tile_pool(name="xp", bufs=8))
        wts = ctx.enter_context(tc.tile_pool(name="wts", bufs=1))
        singles = ctx.enter_context(tc.tile_pool(name="singles", bufs=1))
        etsp = ctx.enter_context(tc.tile_pool(name="etsp", bufs=24))
        nump = ctx.enter_context(tc.tile_pool(name="nump", bufs=4))
        rcp = ctx.enter_context(tc.tile_pool(name="rcp", bufs=4))
        outp = ctx.enter_context(tc.tile_pool(name="outp", bufs=4))
        # PSUM: scores triple-buffer (2-deep leaves ~1us/pair ACT idle on
        # the slot-free round-trip) + two 2KB slots = exactly
        # 16KB/partition. During the x DMA stream all 16KB hold the 4
        # ci-paced q/k groups; in window 3 the triple-buffer carries the
        # pair-1 projection instead of scores.
        ps_sc = ctx.enter_context(tc.tile_pool(name="ps_sc", bufs=2, space="PSUM"))
        ps_sm = ctx.enter_context(tc.tile_pool(name="ps_sm", bufs=4, space="PSUM"))

        # ---- input DMAs: per-ci (wq, wk, x) so a1 consumes chunks as they
        # arrive; wv/wp/ident stream after x (not needed until later).
        wq_src = wqT.rearrange("(a p) d -> p a d", p=P)
        wk_src = wkT.rearrange("(a p) d -> p a d", p=P)
        wv_src = wvT.rearrange("(a p) d -> p a d", p=P)
        wq_sb = wts.tile([P, 8, DH], BF16, name="wq_sb", tag="wq")
        wk_sb = wts.tile([P, 8, DH], BF16, name="wk_sb", tag="wk")
        wv_sb = wts.tile([P, 8, DH], BF16, name="wv_sb", tag="wv")
        xt = [xp.tile([P, N], BF16, name=f"xt{ci}", tag="xt") for ci in range(8)]
        for ci in range(8):
            nc.sync.dma_start(out=wq_sb[:, ci, :], in_=wq_src[:, ci, :])
            nc.gpsimd.dma_start(out=wk_sb[:, ci, :], in_=wk_src[:, ci, :])
            eng = nc.sync if ci % 2 == 0 else nc.gpsimd
            eng.dma_start(out=xt[ci], in_=xT[ci * P:(ci + 1) * P, :])
        for ci in range(8):
            eng = nc.sync if ci % 2 == 0 else nc.gpsimd
            eng.dma_start(out=wv_sb[:, ci, :], in_=wv_src[:, ci, :])
        wp_sb = wts.tile([P, 2, C], BF16, name="wp_sb", tag="wp")
        wp_src = wpT.rearrange("(a p) d -> p a d", p=P)
        nc.sync.dma_start(out=wp_sb[:, 0, :], in_=wp_src[:, 0, :])
        nc.gpsimd.dma_start(out=wp_sb[:, 1, :], in_=wp_src[:, 1, :])
        id_sb = singles.tile([P, P], BF16, name="id_sb", tag="id")
        nc.sync.dma_start(out=id_sb, in_=ident)

        # Pre-trigger the ~2.7us exp table load while DMAs stream.
        dm = singles.tile([1, 2], F32, name="dm", tag="dm")
        nc.vector.memset(dm[:, 0:1], 0.0)
        nc.scalar.activation(out=dm[:, 1:2], in_=dm[:, 0:1], func=Exp, scale=1.0)

        # v tiles: [tokens, head, d+1]; col 64 = ones (denominator trick)
        v_sb = [singles.tile([P, HPC, D + 1], BF16, name=f"v{j}", tag=f"v{j}")
                for j in range(16)]
        for j in range(16):
            nc.gpsimd.memset(v_sb[j][:, :, D:D + 1], 1.0)

        qt = [singles.tile([P, NQ], F32R, name=f"qt{p}", tag=f"qt{p}") for p in range(2)]
        kt = [singles.tile([P, N], F32R, name=f"kt{p}", tag=f"kt{p}") for p in range(2)]

        # ---- a1: q(pair0), k(pair0) both halves, and q(pair1) accumulate
        # ci-paced in all 16KB of PSUM while the x chunks stream in. Only
        # k(pair1) (needed first by scores(2), two exp-windows later) is
        # left for the head-0/1 windows.
        ps_q0 = ps_sc.tile([P, NQ], F32, name="ps_q0", tag="psc")
        ps_k0a = ps_sc.tile([P, NQ], F32, name="ps_k0a", tag="psc")
        ps_k0b = [ps_sm.tile([P, 512], F32, name=f"ps_k0b_{i}", tag="pss")
                  for i in range(2)]
        ps_q1 = [ps_sm.tile([P, 512], F32, name=f"ps_q1_{i}", tag="pss")
                 for i in range(2)]

        def a1_ci(ci, which):
            ss = dict(start=(ci == 0), stop=(ci == 7), skip_group_check=True)
            for nh in range(2):
                sl = slice(nh * 512, (nh + 1) * 512)
                if which in ("qk0", "all"):
                    mm(ps_q0[:, sl], wq_sb[:, ci, 0:P], xt[ci][:, sl], **ss)
                    mm(ps_k0a[:, sl], wk_sb[:, ci, 0:P], xt[ci][:, sl], **ss)
                if which in ("rest", "all"):
                    mm(ps_k0b[nh], wk_sb[:, ci, 0:P],
                       xt[ci][:, 1024 + nh * 512:1024 + (nh + 1) * 512], **ss)
                    mm(ps_q1[nh], wq_sb[:, ci, P:2 * P], xt[ci][:, sl], **ss)

        # ci 7 only runs the two groups the first scores need before the
        # evacuations; its other half drains behind the first exps.
        for ci in range(7):
            a1_ci(ci, "all")
        a1_ci(7, "qk0")
        # parallel evac: q0 on DVE, k0 first half on Pool, so scores(0,0)
        # can issue after ~0.6us of copies.
        nc.vector.tensor_copy(qt[0][:, 0:512], ps_q0[:, 0:512])
        nc.scalar.copy(kt[0][:, 0:512], ps_k0a[:, 0:512])
        nc.vector.tensor_copy(qt[0][:, 512:1024], ps_q0[:, 512:1024])
        nc.scalar.copy(kt[0][:, 512:1024], ps_k0a[:, 512:1024])

        # ---- scores/exp helpers ------------------------------------------
        # Three heads of ets tiles are live at once (consume h-1, feed
        # attnv h, write h+1): ets[0]/ets[1]/ets[3] use the 16-slot pool,
        # ets[2] reuses the xt slots (same 4KB; xt's last readers are the
        # k(pair1) matmuls at the end of window 0).
        ets = {}

        def alloc_ets(h):
            pool, tag = (xp, "xt") if h == 2 else (etsp, "ets")
            ets[h] = [pool.tile([P, 2, NQ], BF16, name=f"et{h}_{k}", tag=tag)
                      for k in range(8)]

        def scores_j(h, j):
            pair, po = h // 2, D * (h % 2)
            ps = ps_sc.tile([P, NQ], F32, name=f"ps_s{h}_{j}", tag="psc")
            lw = kt[pair][po:po + D, j * P:(j + 1) * P]
            for nh in range(2):
                mm(ps[:, nh * 512:(nh + 1) * 512], lw,
                   qt[pair][po:po + D, nh * 512:(nh + 1) * 512],
                   start=True, stop=True)
            nc.scalar.activation(out=ets[h][j // 2][:, j % 2, :], in_=ps,
                                 func=Exp, scale=SCALE)

        # v projection: one 8-step psum group per token chunk
        def v_group(j):
            ps = ps_sm.tile([P, DH], F32, name=f"ps_v{j}", tag="pss")
            for ci in range(8):
                mm(ps, xt[ci][:, j * P:(j + 1) * P], wv_sb[:, ci, :],
                   start=(ci == 0), stop=(ci == 7), skip_group_check=True)
            nc.vector.tensor_copy(
                v_sb[j][:, :, 0:D], ps.rearrange("p (h d) -> p h d", h=HPC))

        # ---- B0 prelude: exp(0) paces everything; the v-groups ride
        # along as PE filler and the leftover a1 work drains behind the
        # first two exps.
        alloc_ets(0)
        scores_j(0, 0)
        scores_j(0, 1)
        a1_ci(7, "rest")
        nc.vector.tensor_copy(kt[0][:, 1024:1536], ps_k0b[0])
        nc.vector.tensor_copy(kt[0][:, 1536:2048], ps_k0b[1])
        for nh in range(2):
            nc.vector.tensor_copy(qt[1][:, nh * 512:(nh + 1) * 512], ps_q1[nh])
        for j in range(2, 16):
            if j - 2 < 11:
                v_group(j - 2)
            scores_j(0, j)

        # ---- stage B: attnv(h) [q,d]-oriented + scores(h+1), per q-tile --
        pp_sb = [singles.tile([P, NQ], BF16, name=f"pp{p}", tag=f"pp{p}")
                 for p in range(2)]
        pnap = [singles.tile([P, D + 1], F32, name=f"pna{q}", tag=f"pna{q}")
                for q in range(8)]

        def attnv_ps(h, q, j0, j1):
            ps = ps_sm.tile([P, D + 1], F32, name=f"ps_a{h}_{q}_{j0}", tag="pss")
            for j in range(j0, j1):
                mm(ps, ets[h][j // 2][:, j % 2, q * P:(q + 1) * P],
                   v_sb[j][:, h, :],
                   start=(j == j0), stop=(j == j1 - 1), skip_group_check=True)
            return ps

        def norm_nm(h, q, src):
            rc = rcp.tile([P, 1], F32, name=f"rc{h}_{q}", tag="rc")
            nc.vector.reciprocal(rc, src[:, D:D + 1])
            nm = nump.tile([P, D], BF16, name=f"nm{h}_{q}", tag="nm")
            nc.vector.tensor_scalar_mul(nm, src[:, 0:D], rc)
            return nm

        def attnv(h, q):
            return norm_nm(h, q, attnv_ps(h, q, 0, 16))

        def transp(h, q, nm):
            po = D * (h % 2)
            tp = ps_sm.tile([P, P], BF16, name=f"tp{h}_{q}", tag="pss")
            nc.tensor.transpose(tp[po:po + D, :], nm, id_sb)
            nc.vector.tensor_copy(
                pp_sb[h // 2][po:po + D, q * P:(q + 1) * P], tp[po:po + D, :])

        cp_dve = nc.vector.tensor_copy
        cp_act = nc.scalar.copy
        cp_pool = nc.gpsimd.tensor_copy

        def proj(pair, q, out_dram, pool, fin_eng):
            # one head-pair's partial projection; pair 0 completes an
            # exp-window before pair 1, so its 1MB of output DMA streams
            # during the exp(3) window instead of after the last exp.
            # One full-row DMA per q-tile: contiguous 2KB destination rows
            # cost half of two strided half-row transfers, and the queues
            # alternate so neither SP-SEQ nor the Pool-side SWDGE trigger
            # (~1us of Pool engine each) serializes the drain.
            pst = [pool.tile([P, 512], F32, name=f"ps_f{pair}_{q}_{i}",
                             tag="psc" if pool is ps_sc else "pss")
                   for i in range(2)]
            for nh in range(2):
                mm(pst[nh], pp_sb[pair][:, q * P:(q + 1) * P],
                   wp_sb[:, pair, nh * 512:(nh + 1) * 512],
                   start=True, stop=True)
            fin = outp.tile([P, C], BF16, name=f"fin{pair}_{q}", tag="fin")
            if fin_eng[0] is fin_eng[1]:
                # single engine: one wide copy amortizes the access setup
                fin_eng[0](fin[:, 0:512], pst[0])
                fin_eng[0](fin[:, 512:1024], pst[1])
            else:
                fin_eng[0](fin[:, 0:512], pst[0])
                fin_eng[1](fin[:, 512:1024], pst[1])
            dma = nc.sync if q % 2 == 0 else nc.gpsimd
            dma.dma_start(out=out_dram[q * P:(q + 1) * P, :], in_=fin)

        # Emission-window h executes during the exp(h+1) ACT window (the
        # PE FIFO self-paces on the 2-deep scores rotation), so attnv(h')
        # lands two windows after its scores and only attnv(3) + the
        # pair-1 projection follow the last exp:
        #   win0: scores(1) + v(12..15) + k1h0 ci-paced filler + k1h1
        #   win1: scores(2) + attnv(0) [q<4] + attnv(1) [q>=4]
        #   win2: scores(3) + attnv(2) [q<4] + proj_a + out_a DMA
        #   win3: attnv(3) riding the exp(3) tail + proj_b + out_b DMA
        pend = []

        def flush(keep):
            while len(pend) > keep:
                transp(*pend.pop(0))

        k1b_blocks = []

        def k1b_block(half):
            t = ps_sm.tile([P, 512], F32, name=f"ps_k1h1_{half}", tag="pss")
            for ci in range(8):
                mm(t, wk_sb[:, ci, P:2 * P],
                   xt[ci][:, 1024 + half * 512:1024 + (half + 1) * 512],
                   start=(ci == 0), stop=(ci == 7), skip_group_check=True)
            cp_dve(kt[1][:, 1024 + half * 512:1024 + (half + 1) * 512], t)

        for h in range(HPC):
            k1 = None
            if h == 1:
                k1b_block(1)
            if h == 0:
                for j in range(11, 16):
                    v_group(j)
                k1 = [ps_sm.tile([P, 512], F32, name=f"ps_k1h0_{i}", tag="pss")
                      for i in range(2)]
            if h < HPC - 1:
                alloc_ets(h + 1)
            for q in range(8):
                if h < HPC - 1:
                    scores_j(h + 1, 2 * q)
                    scores_j(h + 1, 2 * q + 1)
                if k1 is not None:
                    for half in range(2):
                        mm(k1[half], wk_sb[:, q, P:2 * P],
                           xt[q][:, half * 512:(half + 1) * 512],
                           start=(q == 0), stop=(q == 7),
                           skip_group_check=True)
                # deep lag (flush 3, proj lag 3) keeps every non-scores
                # instruction dep-satisfied when PE reaches it, so only
                # the self-pacing scores pairs occupy the 4-deep wait
                # queue and the filler work runs in the exp-stream slack
                if h == 1:
                    hh, qq0 = (0, 2 * q) if q < 4 else (1, 2 * (q - 4))
                    for qq in (qq0, qq0 + 1):
                        pend.append((hh, qq, attnv(hh, qq)))
                        flush(3)
                elif h == 2:
                    pend.append((2, q, attnv(2, q)))
                    flush(3)
                    if q >= 4:
                        # first-half attnv(3) groups (keys 0..1023): their
                        # exp(3,0..7) deps are satisfied by mid-window, so
                        # they fill win-2's PE slack; parking the partials
                        # in SBUF leaves only the 8-matmul second halves
                        # and their chains after the last exp
                        for qq in (2 * (q - 4), 2 * (q - 4) + 1):
                            cp_dve(pnap[qq], attnv_ps(3, qq, 0, 8))
                    if q >= 3:
                        proj(0, q - 3, out_a, ps_sm, (cp_dve, cp_dve))
            if h == 2:
                for q in range(5, 8):
                    proj(0, q, out_a, ps_sm, (cp_dve, cp_dve))
            if k1 is not None:
                # k1h0 evac, then k1h1 reuses the freed slots (its 16
                # matmuls run in window 1's PE slack, before attnv(0))
                for half in range(2):
                    cp_dve(kt[1][:, half * 512:(half + 1) * 512], k1[half])
                k1b_block(0)
        # win-3 tail: second-half attnv(3), combine with the parked first
        # halves, transpose, pair-1 projection, fins on the idle ACT + DVE
        flush(0)
        for q in range(8):
            ps = attnv_ps(3, q, 8, 16)
            tmp = nump.tile([P, D + 1], F32, name=f"tmp{q}", tag="tmp", bufs=2)
            nc.vector.tensor_add(tmp, pnap[q], ps)
            pend.append((3, q, norm_nm(3, q, tmp)))
            flush(2)
            if q >= 2:
                proj(1, q - 2, out_b, ps_sc, (cp_act, cp_dve))
        flush(0)
        for q in range(6, 8):
            proj(1, q, out_b, ps_sc, (cp_act, cp_dve))


def _get_nc():
    if "nc" not in _CACHE:
        _CACHE["nc"] = _build()
    return _CACHE["nc"]


def kernel(x, wq, wk, wv, w_proj, b_proj):
    bf = ml_dtypes.bfloat16
    x = np.asarray(x, dtype=np.float32)
    wq = np.asarray(wq, dtype=np.float32)
    wk = np.asarray(wk, dtype=np.float32)
    wv = np.asarray(wv, dtype=np.float32)
    w_proj = np.asarray(w_proj, dtype=np.float32)
    b_proj = np.asarray(b_proj, dtype=np.float32)
    ident = np.eye(P, dtype=bf)

    nc = _get_nc()
    in_maps = []
    for core in range(8):
        b, g = divmod(core, 4)
        sl = slice(g * DH, (g + 1) * DH)
        in_maps.append({
            "xT": np.ascontiguousarray(x[b].T.astype(bf)),
            "wqT": np.ascontiguousarray(wq[sl, :].T.astype(bf)),
            "wkT": np.ascontiguousarray(wk[sl, :].T.astype(bf)),
            "wvT": np.ascontiguousarray(wv[sl, :].T.astype(bf)),
            "wpT": np.ascontiguousarray(w_proj[:, sl].T.astype(bf)),
            "ident": ident,
        })

    res = run_bass_kernel_spmd(nc, in_maps, core_ids=list(range(8)),
                               trace=bool(int(os.environ.get("KERNEL_TRACE", "0"))))
    _CACHE["last_results"] = res
    outs = [res.results[c]["out_a"].astype(np.float32)
            + res.results[c]["out_b"].astype(np.float32) for c in range(8)]
    full = np.stack([outs[0] + outs[1] + outs[2] + outs[3],
                     outs[4] + outs[5] + outs[6] + outs[7]])
    full += b_proj[None, None, :]
    return full.astype(np.float32)


# revision 43
# speedup vs baseline: 1.0143x; 1.0143x over previous
"""Cross-attention kernel for Trainium2, 8-core SPMD.

Problem (all fp32):
  x [2, 2048, 1024]; wq/wk/wv/w_proj [1024, 1024]; b_proj [1024]
  q = x[:, :1024] @ wq.T   (16 heads x 64)
  k, v = x @ wk.T, x @ wv.T
  out = softmax(q k^T / 8) v  -> proj + bias  -> [2, 1024, 1024]

Sharding: 8 cores = 2 (batch) x 4 (head-groups of 4 heads). Each core
computes its batch's QKV for its 4 heads, full attention for those heads,
and a partial projection (its 256 contraction rows of w_proj). Host sums
the 4 bf16 partials per batch in fp32 and adds the bias.

Numerics: all DRAM inputs are bf16 (halves the input DMA, which bounds
how early the exp stream can start); q/k and the scores matmul stay in
fp32(r); exp(scores), v, attention output and the projection run in bf16
(validated 5.4e-3 max rel err vs the 2e-2 gate).

Layouts: x is kept feature-on-partition (xT [c, n]); q/k are produced
transposed (qT/kT [dh_pair, n]); v natural [n, d] with an appended
ones-column so attn@v also emits the softmax denominator. attn@v is
oriented [q, d] (queries on partitions): the output free size is 65
instead of 512, which halves its PE cost since PE time is
output-free-size * steps. The per-(head, q-tile) [128, 65] PSUM result is
normalized during evacuation (reciprocal of the denominator column +
per-partition tensor_scalar broadcast), PE-transposed into a packed
[2-heads x 64, q] tile via tile_position, and the projection then runs
with full 128-row stationary operands (half the naive cost).

Schedule: the ACT engine's exp stream (~66us: 64 x [128,1024] exps) is
the stage-B pacer; scores(h+1) and attnv(h) interleave per q-tile so ACT
never starves, and the projection pipelines per-q-tile inside head 3's
exp window. Stage A is paced by the 5MB x+wq+wk DMA stream (~19us);
scores(0)/exp(0) start immediately after the last x chunk, with the
remaining q/k/v work used as PE filler between them.
"""

import os
import numpy as np
import ml_dtypes

import concourse.bacc as bacc
import concourse.bass as bass
import concourse.tile as tile
import concourse.mybir as mybir
from concourse.bass_utils import run_bass_kernel_spmd

F32 = mybir.dt.float32
F32R = mybir.dt.float32r
BF16 = mybir.dt.bfloat16

C = 1024          # model dim
N = 2048          # kv tokens
NQ = 1024         # query tokens
HPC = 4           # heads per core
D = 64            # head dim
DH = HPC * D      # per-core slice of C (256)
SCALE = D ** -0.5
P = 128

_CACHE: dict = {}


def _build():
    nc = bacc.Bacc("TRN2", target_bir_lowering=False, debug=False, num_devices=8)

    xT = nc.dram_tensor("xT", [C, N], BF16, kind="ExternalInput").ap()
    wqT = nc.dram_tensor("wqT", [C, DH], BF16, kind="ExternalInput").ap()
    wkT = nc.dram_tensor("wkT", [C, DH], BF16, kind="ExternalInput").ap()
    wvT = nc.dram_tensor("wvT", [C, DH], BF16, kind="ExternalInput").ap()
    wpT = nc.dram_tensor("wpT", [DH, C], BF16, kind="ExternalInput").ap()
    ident = nc.dram_tensor("ident", [P, P], BF16, kind="ExternalInput").ap()
    out_a = nc.dram_tensor("out_a", [NQ, C], BF16, kind="ExternalOutput").ap()
    out_b = nc.dram_tensor("out_b", [NQ, C], BF16, kind="ExternalOutput").ap()

    with tile.TileContext(nc) as tc, \
            nc.allow_low_precision(reason="bf16/fp32r attention pipeline, validated 5.4e-3 rel err"):
        _emit(tc, xT, wqT, wkT, wvT, wpT, ident, out_a, out_b)

    nc.compile()
    return nc


def _emit(tc, xT, wqT, wkT, wvT, wpT, ident, out_a, out_b):
    nc = tc.nc
    mm = nc.tensor.matmul
    Exp = mybir.ActivationFunctionType.Exp

    from contextlib import ExitStack

    with ExitStack() as ctx:
        xp = ctx.enter_context(tc.tile_pool(name="xp", bufs=8))
        wts = ctx.enter_context(tc.tile_pool(name="wts", bufs=1))
        singles = ctx.enter_context(tc.tile_pool(name="singles", bufs=1))
        etsp = ctx.enter_context(tc.tile_pool(name="etsp", bufs=24))
        nump = ctx.enter_context(tc.tile_pool(name="nump", bufs=4))
        rcp = ctx.enter_context(tc.tile_pool(name="rcp", bufs=4))
        outp = ctx.enter_context(tc.tile_pool(name="outp", bufs=4))
        # PSUM: scores triple-buffer (2-deep leaves ~1us/pair ACT idle on
        # the slot-free round-trip) + two 2KB slots = exactly
        # 16KB/partition. During the x DMA stream all 16KB hold the 4
        # ci-paced q/k groups; in window 3 the triple-buffer carries the
        # pair-1 projection instead of scores.
        ps_sc = ctx.enter_context(tc.tile_pool(name="ps_sc", bufs=2, space="PSUM"))
        ps_sm = ctx.enter_context(tc.tile_pool(name="ps_sm", bufs=4, space="PSUM"))

        # ---- input DMAs: per-ci (wq, wk, x) so a1 consumes chunks as they
        # arrive; wv/wp/ident stream after x (not needed until later).
        wq_src = wqT.rearrange("(a p) d -> p a d", p=P)
        wk_src = wkT.rearrange("(a p) d -> p a d", p=P)
        wv_src = wvT.rearrange("(a p) d -> p a d", p=P)
        wq_sb = wts.tile([P, 8, DH], BF16, name="wq_sb", tag="wq")
        wk_sb = wts.tile([P, 8, DH], BF16, name="wk_sb", tag="wk")
        wv_sb = wts.tile([P, 8, DH], BF16, name="wv_sb", tag="wv")
        xt = [xp.tile([P, N], BF16, name=f"xt{ci}", tag="xt") for ci in range(8)]
        for ci in range(8):
            nc.sync.dma_start(out=wq_sb[:, ci, :], in_=wq_src[:, ci, :])
            nc.gpsimd.dma_start(out=wk_sb[:, ci, :], in_=wk_src[:, ci, :])
            eng = nc.sync if ci % 2 == 0 else nc.gpsimd
            eng.dma_start(out=xt[ci], in_=xT[ci * P:(ci + 1) * P, :])
        for ci in range(8):
            eng = nc.sync if ci % 2 == 0 else nc.gpsimd
            eng.dma_start(out=wv_sb[:, ci, :], in_=wv_src[:, ci, :])
        wp_sb = wts.tile([P, 2, C], BF16, name="wp_sb", tag="wp")
        wp_src = wpT.rearrange("(a p) d -> p a d", p=P)
        nc.sync.dma_start(out=wp_sb[:, 0, :], in_=wp_src[:, 0, :])
        nc.gpsimd.dma_start(out=wp_sb[:, 1, :], in_=wp_src[:, 1, :])
        id_sb = singles.tile([P, P], BF16, name="id_sb", tag="id")
        nc.sync.dma_start(out=id_sb, in_=ident)

        # Pre-trigger the ~2.7us exp table load while DMAs stream.
        dm = singles.tile([1, 2], F32, name="dm", tag="dm")
        nc.vector.memset(dm[:, 0:1], 0.0)
        nc.scalar.activation(out=dm[:, 1:2], in_=dm[:, 0:1], func=Exp, scale=1.0)

        # v tiles: [tokens, head, d+1]; col 64 = ones (denominator trick)
        v_sb = [singles.tile([P, HPC, D + 1], BF16, name=f"v{j}", tag=f"v{j}")
                for j in range(16)]
        for j in range(16):
            nc.gpsimd.memset(v_sb[j][:, :, D:D + 1], 1.0)

        qt = [singles.tile([P, NQ], F32R, name=f"qt{p}", tag=f"qt{p}") for p in range(2)]
        kt = [singles.tile([P, N], F32R, name=f"kt{p}", tag=f"kt{p}") for p in range(2)]

        # ---- a1: q(pair0), k(pair0) both halves, and q(pair1) accumulate
        # ci-paced in all 16KB of PSUM while the x chunks stream in. Only
        # k(pair1) (needed first by scores(2), two exp-windows later) is
        # left for the head-0/1 windows.
        ps_q0 = ps_sc.tile([P, NQ], F32, name="ps_q0", tag="psc")
        ps_k0a = ps_sc.tile([P, NQ], F32, name="ps_k0a", tag="psc")
        ps_k0b = [ps_sm.tile([P, 512], F32, name=f"ps_k0b_{i}", tag="pss")
                  for i in range(2)]
        ps_q1 = [ps_sm.tile([P, 512], F32, name=f"ps_q1_{i}", tag="pss")
                 for i in range(2)]

        def a1_ci(ci, which):
            ss = dict(start=(ci == 0), stop=(ci == 7), skip_group_check=True)
            for nh in range(2):
                sl = slice(nh * 512, (nh + 1) * 512)
                if which in ("qk0", "all"):
                    mm(ps_q0[:, sl], wq_sb[:, ci, 0:P], xt[ci][:, sl], **ss)
                    mm(ps_k0a[:, sl], wk_sb[:, ci, 0:P], xt[ci][:, sl], **ss)
                if which in ("rest", "all"):
                    mm(ps_k0b[nh], wk_sb[:, ci, 0:P],
                       xt[ci][:, 1024 + nh * 512:1024 + (nh + 1) * 512], **ss)
                    mm(ps_q1[nh], wq_sb[:, ci, P:2 * P], xt[ci][:, sl], **ss)

        # ci 7 only runs the two groups the first scores need before the
        # evacuations; its other half drains behind the first exps.
        for ci in range(7):
            a1_ci(ci, "all")
        a1_ci(7, "qk0")
        # parallel evac: q0 on DVE, k0 first half on Pool, so scores(0,0)
        # can issue after ~0.6us of copies.
        nc.vector.tensor_copy(qt[0][:, 0:512], ps_q0[:, 0:512])
        nc.scalar.copy(kt[0][:, 0:512], ps_k0a[:, 0:512])
        nc.vector.tensor_copy(qt[0][:, 512:1024], ps_q0[:, 512:1024])
        nc.scalar.copy(kt[0][:, 512:1024], ps_k0a[:, 512:1024])

        # ---- scores/exp helpers ------------------------------------------
        # Three heads of ets tiles are live at once (consume h-1, feed
        # attnv h, write h+1): ets[0]/ets[1]/ets[3] use the 16-slot pool,
        # ets[2] reuses the xt slots (same 4KB; xt's last readers are the
        # k(pair1) matmuls at the end of window 0).
        ets = {}

        def alloc_ets(h):
            pool, tag = (xp, "xt") if h == 2 else (etsp, "ets")
            ets[h] = [pool.tile([P, 2, NQ], BF16, name=f"et{h}_{k}", tag=tag)
                      for k in range(8)]

        def scores_j(h, j):
            pair, po = h // 2, D * (h % 2)
            ps = ps_sc.tile([P, NQ], F32, name=f"ps_s{h}_{j}", tag="psc")
            lw = kt[pair][po:po + D, j * P:(j + 1) * P]
            for nh in range(2):
                mm(ps[:, nh * 512:(nh + 1) * 512], lw,
                   qt[pair][po:po + D, nh * 512:(nh + 1) * 512],
                   start=True, stop=True)
            nc.scalar.activation(out=ets[h][j // 2][:, j % 2, :], in_=ps,
                                 func=Exp, scale=SCALE)

        # v projection: one 8-step psum group per token chunk
        def v_group(j):
            ps = ps_sm.tile([P, DH], F32, name=f"ps_v{j}", tag="pss")
            for ci in range(8):
                mm(ps, xt[ci][:, j * P:(j + 1) * P], wv_sb[:, ci, :],
                   start=(ci == 0), stop=(ci == 7), skip_group_check=True)
            nc.vector.tensor_copy(
                v_sb[j][:, :, 0:D], ps.rearrange("p (h d) -> p h d", h=HPC))

        # ---- B0 prelude: exp(0) paces everything; the v-groups ride
        # along as PE filler and the leftover a1 work drains behind the
        # first two exps.
        alloc_ets(0)
        scores_j(0, 0)
        scores_j(0, 1)
        a1_ci(7, "rest")
        nc.vector.tensor_copy(kt[0][:, 1024:1536], ps_k0b[0])
        nc.vector.tensor_copy(kt[0][:, 1536:2048], ps_k0b[1])
        for nh in range(2):
            nc.vector.tensor_copy(qt[1][:, nh * 512:(nh + 1) * 512], ps_q1[nh])
        for j in range(2, 16):
            if j - 2 < 11:
                v_group(j - 2)
            scores_j(0, j)

        # ---- stage B: attnv(h) [q,d]-oriented + scores(h+1), per q-tile --
        pp_sb = [singles.tile([P, NQ], BF16, name=f"pp{p}", tag=f"pp{p}")
                 for p in range(2)]
        pnap = [singles.tile([P, D + 1], F32, name=f"pna{q}", tag=f"pna{q}")
                for q in range(8)]

        def attnv_ps(h, q, j0, j1):
            ps = ps_sm.tile([P, D + 1], F32, name=f"ps_a{h}_{q}_{j0}", tag="pss")
            for j in range(j0, j1):
                mm(ps, ets[h][j // 2][:, j % 2, q * P:(q + 1) * P],
                   v_sb[j][:, h, :],
                   start=(j == j0), stop=(j == j1 - 1), skip_group_check=True)
            return ps

        def norm_nm(h, q, src):
            rc = rcp.tile([P, 1], F32, name=f"rc{h}_{q}", tag="rc")
            nc.vector.reciprocal(rc, src[:, D:D + 1])
            nm = nump.tile([P, D], BF16, name=f"nm{h}_{q}", tag="nm")
            nc.vector.tensor_scalar_mul(nm, src[:, 0:D], rc)
            return nm

        def attnv(h, q):
            return norm_nm(h, q, attnv_ps(h, q, 0, 16))

        def transp(h, q, nm):
            po = D * (h % 2)
            tp = ps_sm.tile([P, P], BF16, name=f"tp{h}_{q}", tag="pss")
            nc.tensor.transpose(tp[po:po + D, :], nm, id_sb)
            nc.vector.tensor_copy(
                pp_sb[h // 2][po:po + D, q * P:(q + 1) * P], tp[po:po + D, :])

        cp_dve = nc.vector.tensor_copy
        cp_act = nc.scalar.copy
        cp_pool = nc.gpsimd.tensor_copy

        def proj(pair, q, out_dram, pool, fin_eng):
            # one head-pair's partial projection; pair 0 completes an
            # exp-window before pair 1, so its 1MB of output DMA streams
            # during the exp(3) window instead of after the last exp.
            # One full-row DMA per q-tile: contiguous 2KB destination rows
            # cost half of two strided half-row transfers, and the queues
            # alternate so neither SP-SEQ nor the Pool-side SWDGE trigger
            # (~1us of Pool engine each) serializes the drain.
            pst = [pool.tile([P, 512], F32, name=f"ps_f{pair}_{q}_{i}",
                             tag="psc" if pool is ps_sc else "pss")
                   for i in range(2)]
            for nh in range(2):
                mm(pst[nh], pp_sb[pair][:, q * P:(q + 1) * P],
                   wp_sb[:, pair, nh * 512:(nh + 1) * 512],
                   start=True, stop=True)
            fin = outp.tile([P, C], BF16, name=f"fin{pair}_{q}", tag="fin")
            if fin_eng[0] is fin_eng[1]:
                # single engine: one wide copy amortizes the access setup
                fin_eng[0](fin[:, 0:512], pst[0])
                fin_eng[0](fin[:, 512:1024], pst[1])
            else:
                fin_eng[0](fin[:, 0:512], pst[0])
                fin_eng[1](fin[:, 512:1024], pst[1])
            dma = nc.sync if q % 2 == 0 else nc.gpsimd
            dma.dma_start(out=out_dram[q * P:(q + 1) * P, :], in_=fin)

        # Emission-window h executes during the exp(h+1) ACT window (the
        # PE FIFO self-paces on the 2-deep scores rotation), so attnv(h')
        # lands two windows after its scores and only attnv(3) + the
        # pair-1 projection follow the last exp:
        #   win0: scores(1) + v(12..15) + k1h0 ci-paced filler + k1h1
        #   win1: scores(2) + attnv(0) [q<4] + attnv(1) [q>=4]
        #   win2: scores(3) + attnv(2) [q<4] + proj_a + out_a DMA
        #   win3: attnv(3) riding the exp(3) tail + proj_b + out_b DMA
        pend = []

        def flush(keep):
            while len(pend) > keep:
                transp(*pend.pop(0))

        k1b_blocks = []

        def k1b_block(half):
            t = ps_sm.tile([P, 512], F32, name=f"ps_k1h1_{half}", tag="pss")
            for ci in range(8):
                mm(t, wk_sb[:, ci, P:2 * P],
                   xt[ci][:, 1024 + half * 512:1024 + (half + 1) * 512],
                   start=(ci == 0), stop=(ci == 7), skip_group_check=True)
            cp_dve(kt[1][:, 1024 + half * 512:1024 + (half + 1) * 512], t)

        for h in range(HPC):
            k1 = None
            if h == 0:
                k1 = [ps_sm.tile([P, 512], F32, name=f"ps_k1h0_{i}", tag="pss")
                      for i in range(2)]
            if h < HPC - 1:
                alloc_ets(h + 1)
            for q in range(8):
                if h < HPC - 1:
                    scores_j(h + 1, 2 * q)
                    scores_j(h + 1, 2 * q + 1)
                if h == 0 and q == 1:
                    # the first exp(1) pair is already in flight; now the
                    # v tail rides win-0's PE slack without delaying it
                    for j in range(11, 16):
                        v_group(j)
                if h == 1 and q == 1:
                    k1b_block(0)
                if h == 1 and q == 2:
                    k1b_block(1)
                if k1 is not None:
                    for half in range(2):
                        mm(k1[half], wk_sb[:, q, P:2 * P],
                           xt[q][:, half * 512:(half + 1) * 512],
                           start=(q == 0), stop=(q == 7),
                           skip_group_check=True)
                # deep lag (flush 3, proj lag 3) keeps every non-scores
                # instruction dep-satisfied when PE reaches it, so only
                # the self-pacing scores pairs occupy the 4-deep wait
                # queue and the filler work runs in the exp-stream slack
                if h == 1:
                    hh, qq0 = (0, 2 * q) if q < 4 else (1, 2 * (q - 4))
                    for qq in (qq0, qq0 + 1):
                        pend.append((hh, qq, attnv(hh, qq)))
                        flush(3)
                elif h == 2:
                    pend.append((2, q, attnv(2, q)))
                    flush(3)
                    if q >= 4:
                        # first-half attnv(3) groups (keys 0..1023): their
                        # exp(3,0..7) deps are satisfied by mid-window, so
                        # they fill win-2's PE slack; parking the partials
                        # in SBUF leaves only the 8-matmul second halves
                        # and their chains after the last exp
                        for qq in (2 * (q - 4), 2 * (q - 4) + 1):
                            cp_dve(pnap[qq], attnv_ps(3, qq, 0, 8))
                    if q >= 3:
                        proj(0, q - 3, out_a, ps_sm, (cp_dve, cp_dve))
            if h == 2:
                for q in range(5, 8):
                    proj(0, q, out_a, ps_sm, (cp_dve, cp_dve))
            if k1 is not None:
                # k1h0 evac, then k1h1 reuses the freed slots (its 16
                # matmuls run in window 1's PE slack, before attnv(0))
                for half in range(2):
                    cp_dve(kt[1][:, half * 512:(half + 1) * 512], k1[half])
        # win-3 tail: second-half attnv(3), combine with the parked first
        # halves, transpose, pair-1 projection, fins on the idle ACT + DVE
        flush(0)
        for q in range(8):
            ps = attnv_ps(3, q, 8, 16)
            tmp = nump.tile([P, D + 1], F32, name=f"tmp{q}", tag="tmp", bufs=2)
            nc.vector.tensor_add(tmp, pnap[q], ps)
            pend.append((3, q, norm_nm(3, q, tmp)))
            flush(2)
            if q >= 2:
                proj(1, q - 2, out_b, ps_sc, (cp_act, cp_dve))
        flush(0)
        for q in range(6, 8):
            proj(1, q, out_b, ps_sc, (cp_act, cp_dve))


def _get_nc():
    if "nc" not in _CACHE:
        _CACHE["nc"] = _build()
    return _CACHE["nc"]


def kernel(x, wq, wk, wv, w_proj, b_proj):
    bf = ml_dtypes.bfloat16
    x = np.asarray(x, dtype=np.float32)
    wq = np.asarray(wq, dtype=np.float32)
    wk = np.asarray(wk, dtype=np.float32)
    wv = np.asarray(wv, dtype=np.float32)
    w_proj = np.asarray(w_proj, dtype=np.float32)
    b_proj = np.asarray(b_proj, dtype=np.float32)
    ident = np.eye(P, dtype=bf)

    nc = _get_nc()
    in_maps = []
    for core in range(8):
        b, g = divmod(core, 4)
        sl = slice(g * DH, (g + 1) * DH)
        in_maps.append({
            "xT": np.ascontiguousarray(x[b].T.astype(bf)),
            "wqT": np.ascontiguousarray(wq[sl, :].T.astype(bf)),
            "wkT": np.ascontiguousarray(wk[sl, :].T.astype(bf)),
            "wvT": np.ascontiguousarray(wv[sl, :].T.astype(bf)),
            "wpT": np.ascontiguousarray(w_proj[:, sl].T.astype(bf)),
            "ident": ident,
        })

    res = run_bass_kernel_spmd(nc, in_maps, core_ids=list(range(8)),
                               trace=bool(int(os.environ.get("KERNEL_TRACE", "0"))))
    _CACHE["last_results"] = res
    outs = [res.results[c]["out_a"].astype(np.float32)
            + res.results[c]["out_b"].astype(np.float32) for c in range(8)]
    full = np.stack([outs[0] + outs[1] + outs[2] + outs[3],
                     outs[4] + outs[5] + outs[6] + outs[7]])
    full += b_proj[None, None, :]
    return full.astype(np.float32)


# revision 46
# speedup vs baseline: 1.0398x; 1.0252x over previous
"""Cross-attention kernel for Trainium2, 8-core SPMD.

Problem (all fp32):
  x [2, 2048, 1024]; wq/wk/wv/w_proj [1024, 1024]; b_proj [1024]
  q = x[:, :1024] @ wq.T   (16 heads x 64)
  k, v = x @ wk.T, x @ wv.T
  out = softmax(q k^T / 8) v  -> proj + bias  -> [2, 1024, 1024]

Sharding: 8 cores = 2 (batch) x 4 (head-groups of 4 heads). Each core
computes its batch's QKV for its 4 heads, full attention for those heads,
and a partial projection (its 256 contraction rows of w_proj). Host sums
the 4 bf16 partials per batch in fp32 and adds the bias.

Numerics: all DRAM inputs are bf16 (halves the input DMA, which bounds
how early the exp stream can start); q/k and the scores matmul stay in
fp32(r); exp(scores), v, attention output and the projection run in bf16
(validated 5.4e-3 max rel err vs the 2e-2 gate).

Layouts: x is kept feature-on-partition (xT [c, n]); q/k are produced
transposed (qT/kT [dh_pair, n]); v natural [n, d] with an appended
ones-column so attn@v also emits the softmax denominator. attn@v is
oriented [q, d] (queries on partitions): the output free size is 65
instead of 512, which halves its PE cost since PE time is
output-free-size * steps. The per-(head, q-tile) [128, 65] PSUM result is
normalized during evacuation (reciprocal of the denominator column +
per-partition tensor_scalar broadcast), PE-transposed into a packed
[2-heads x 64, q] tile via tile_position, and the projection then runs
with full 128-row stationary operands (half the naive cost).

Schedule: the ACT engine's exp stream (~66us: 64 x [128,1024] exps) is
the stage-B pacer; scores(h+1) and attnv(h) interleave per q-tile so ACT
never starves, and the projection pipelines per-q-tile inside head 3's
exp window. Stage A is paced by the 5MB x+wq+wk DMA stream (~19us);
scores(0)/exp(0) start immediately after the last x chunk, with the
remaining q/k/v work used as PE filler between them.
"""

import os
import numpy as np
import ml_dtypes

import concourse.bacc as bacc
import concourse.bass as bass
import concourse.tile as tile
import concourse.mybir as mybir
from concourse.bass_utils import run_bass_kernel_spmd

F32 = mybir.dt.float32
F32R = mybir.dt.float32r
BF16 = mybir.dt.bfloat16

C = 1024          # model dim
N = 2048          # kv tokens
NQ = 1024         # query tokens
HPC = 4           # heads per core
D = 64            # head dim
DH = HPC * D      # per-core slice of C (256)
SCALE = D ** -0.5
P = 128

_CACHE: dict = {}


def _build():
    nc = bacc.Bacc("TRN2", target_bir_lowering=False, debug=False, num_devices=8)

    xT = nc.dram_tensor("xT", [C, N], BF16, kind="ExternalInput").ap()
    wqT = nc.dram_tensor("wqT", [C, DH], BF16, kind="ExternalInput").ap()
    wkT = nc.dram_tensor("wkT", [C, DH], BF16, kind="ExternalInput").ap()
    wvT = nc.dram_tensor("wvT", [C, DH], BF16, kind="ExternalInput").ap()
    wpT = nc.dram_tensor("wpT", [DH, C], BF16, kind="ExternalInput").ap()
    ident = nc.dram_tensor("ident", [P, P], BF16, kind="ExternalInput").ap()
    out_a = nc.dram_tensor("out_a", [NQ, C], BF16, kind="ExternalOutput").ap()
    out_b = nc.dram_tensor("out_b", [NQ, C], BF16, kind="ExternalOutput").ap()

    with tile.TileContext(nc) as tc, \
            nc.allow_low_precision(reason="bf16/fp32r attention pipeline, validated 5.4e-3 rel err"):
        _emit(tc, xT, wqT, wkT, wvT, wpT, ident, out_a, out_b)

    nc.compile()
    return nc


def _emit(tc, xT, wqT, wkT, wvT, wpT, ident, out_a, out_b):
    nc = tc.nc
    mm = nc.tensor.matmul
    Exp = mybir.ActivationFunctionType.Exp

    from contextlib import ExitStack

    with ExitStack() as ctx:
        xp = ctx.enter_context(tc.tile_pool(name="xp", bufs=8))
        wts = ctx.enter_context(tc.tile_pool(name="wts", bufs=1))
        singles = ctx.enter_context(tc.tile_pool(name="singles", bufs=1))
        etsp = ctx.enter_context(tc.tile_pool(name="etsp", bufs=24))
        nump = ctx.enter_context(tc.tile_pool(name="nump", bufs=4))
        rcp = ctx.enter_context(tc.tile_pool(name="rcp", bufs=4))
        outp = ctx.enter_context(tc.tile_pool(name="outp", bufs=4))
        # PSUM: scores triple-buffer (2-deep leaves ~1us/pair ACT idle on
        # the slot-free round-trip) + two 2KB slots = exactly
        # 16KB/partition. During the x DMA stream all 16KB hold the 4
        # ci-paced q/k groups; in window 3 the triple-buffer carries the
        # pair-1 projection instead of scores.
        ps_sc = ctx.enter_context(tc.tile_pool(name="ps_sc", bufs=2, space="PSUM"))
        ps_sm = ctx.enter_context(tc.tile_pool(name="ps_sm", bufs=4, space="PSUM"))

        # ---- input DMAs: per-ci (wq, wk, x) so a1 consumes chunks as they
        # arrive; wv/wp/ident stream after x (not needed until later).
        wq_src = wqT.rearrange("(a p) d -> p a d", p=P)
        wk_src = wkT.rearrange("(a p) d -> p a d", p=P)
        wv_src = wvT.rearrange("(a p) d -> p a d", p=P)
        wq_sb = wts.tile([P, 8, DH], BF16, name="wq_sb", tag="wq")
        wk_sb = wts.tile([P, 8, DH], BF16, name="wk_sb", tag="wk")
        wv_sb = wts.tile([P, 8, DH], BF16, name="wv_sb", tag="wv")
        xt = [xp.tile([P, N], BF16, name=f"xt{ci}", tag="xt") for ci in range(8)]
        for ci in range(8):
            nc.sync.dma_start(out=wq_sb[:, ci, :], in_=wq_src[:, ci, :])
            nc.gpsimd.dma_start(out=wk_sb[:, ci, :], in_=wk_src[:, ci, :])
            eng = nc.sync if ci % 2 == 0 else nc.gpsimd
            eng.dma_start(out=xt[ci], in_=xT[ci * P:(ci + 1) * P, :])
        for ci in range(8):
            eng = nc.sync if ci % 2 == 0 else nc.gpsimd
            eng.dma_start(out=wv_sb[:, ci, :], in_=wv_src[:, ci, :])
        wp_sb = wts.tile([P, 2, C], BF16, name="wp_sb", tag="wp")
        wp_src = wpT.rearrange("(a p) d -> p a d", p=P)
        nc.sync.dma_start(out=wp_sb[:, 0, :], in_=wp_src[:, 0, :])
        nc.gpsimd.dma_start(out=wp_sb[:, 1, :], in_=wp_src[:, 1, :])
        id_sb = singles.tile([P, P], BF16, name="id_sb", tag="id")
        nc.sync.dma_start(out=id_sb, in_=ident)

        # Pre-trigger the ~2.7us exp table load while DMAs stream.
        dm = singles.tile([1, 2], F32, name="dm", tag="dm")
        nc.vector.memset(dm[:, 0:1], 0.0)
        nc.scalar.activation(out=dm[:, 1:2], in_=dm[:, 0:1], func=Exp, scale=1.0)

        # v tiles: [tokens, head, d+1]; col 64 = ones (denominator trick)
        v_sb = [singles.tile([P, HPC, D + 1], BF16, name=f"v{j}", tag=f"v{j}")
                for j in range(16)]
        for j in range(16):
            nc.gpsimd.memset(v_sb[j][:, :, D:D + 1], 1.0)

        qt = [singles.tile([P, NQ], F32R, name=f"qt{p}", tag=f"qt{p}") for p in range(2)]
        kt = [singles.tile([P, N], F32R, name=f"kt{p}", tag=f"kt{p}") for p in range(2)]

        # ---- a1: q(pair0), k(pair0) both halves, and q(pair1) accumulate
        # ci-paced in all 16KB of PSUM while the x chunks stream in. Only
        # k(pair1) (needed first by scores(2), two exp-windows later) is
        # left for the head-0/1 windows.
        ps_q0 = ps_sc.tile([P, NQ], F32, name="ps_q0", tag="psc")
        ps_k0a = ps_sc.tile([P, NQ], F32, name="ps_k0a", tag="psc")
        ps_k0b = [ps_sm.tile([P, 512], F32, name=f"ps_k0b_{i}", tag="pss")
                  for i in range(2)]
        ps_q1 = [ps_sm.tile([P, 512], F32, name=f"ps_q1_{i}", tag="pss")
                 for i in range(2)]

        def a1_ci(ci, which):
            ss = dict(start=(ci == 0), stop=(ci == 7), skip_group_check=True)
            for nh in range(2):
                sl = slice(nh * 512, (nh + 1) * 512)
                if which in ("qk0", "all"):
                    mm(ps_q0[:, sl], wq_sb[:, ci, 0:P], xt[ci][:, sl], **ss)
                    mm(ps_k0a[:, sl], wk_sb[:, ci, 0:P], xt[ci][:, sl], **ss)
                if which in ("rest", "all"):
                    mm(ps_k0b[nh], wk_sb[:, ci, 0:P],
                       xt[ci][:, 1024 + nh * 512:1024 + (nh + 1) * 512], **ss)
                    mm(ps_q1[nh], wq_sb[:, ci, P:2 * P], xt[ci][:, sl], **ss)

        # ci 7 only runs the two groups the first scores need before the
        # evacuations; its other half drains behind the first exps.
        for ci in range(7):
            a1_ci(ci, "all")
        a1_ci(7, "qk0")
        # parallel evac: q0 on DVE, k0 first half on Pool, so scores(0,0)
        # can issue after ~0.6us of copies.
        nc.vector.tensor_copy(qt[0][:, 0:512], ps_q0[:, 0:512])
        nc.scalar.copy(kt[0][:, 0:512], ps_k0a[:, 0:512])
        nc.vector.tensor_copy(qt[0][:, 512:1024], ps_q0[:, 512:1024])
        nc.scalar.copy(kt[0][:, 512:1024], ps_k0a[:, 512:1024])

        # ---- scores/exp helpers ------------------------------------------
        # Three heads of ets tiles are live at once (consume h-1, feed
        # attnv h, write h+1): ets[0]/ets[1]/ets[3] use the 16-slot pool,
        # ets[2] reuses the xt slots (same 4KB; xt's last readers are the
        # k(pair1) matmuls at the end of window 0).
        ets = {}

        def alloc_ets(h):
            pool, tag = (xp, "xt") if h == 2 else (etsp, "ets")
            ets[h] = [pool.tile([P, 2, NQ], BF16, name=f"et{h}_{k}", tag=tag)
                      for k in range(8)]

        def scores_j(h, j):
            pair, po = h // 2, D * (h % 2)
            ps = ps_sc.tile([P, NQ], F32, name=f"ps_s{h}_{j}", tag="psc")
            lw = kt[pair][po:po + D, j * P:(j + 1) * P]
            for nh in range(2):
                mm(ps[:, nh * 512:(nh + 1) * 512], lw,
                   qt[pair][po:po + D, nh * 512:(nh + 1) * 512],
                   start=True, stop=True)
            nc.scalar.activation(out=ets[h][j // 2][:, j % 2, :], in_=ps,
                                 func=Exp, scale=SCALE)

        # v projection: one 8-step psum group per token chunk
        def v_group(j):
            ps = ps_sm.tile([P, DH], F32, name=f"ps_v{j}", tag="pss")
            for ci in range(8):
                mm(ps, xt[ci][:, j * P:(j + 1) * P], wv_sb[:, ci, :],
                   start=(ci == 0), stop=(ci == 7), skip_group_check=True)
            nc.vector.tensor_copy(
                v_sb[j][:, :, 0:D], ps.rearrange("p (h d) -> p h d", h=HPC))

        # ---- B0 prelude: exp(0) paces everything; the v-groups ride
        # along as PE filler and the leftover a1 work drains behind the
        # first two exps.
        alloc_ets(0)
        scores_j(0, 0)
        scores_j(0, 1)
        a1_ci(7, "rest")
        nc.vector.tensor_copy(kt[0][:, 1024:1536], ps_k0b[0])
        nc.vector.tensor_copy(kt[0][:, 1536:2048], ps_k0b[1])
        for nh in range(2):
            nc.vector.tensor_copy(qt[1][:, nh * 512:(nh + 1) * 512], ps_q1[nh])
        for j in range(2, 16):
            if j - 2 < 11:
                v_group(j - 2)
            scores_j(0, j)

        # ---- stage B: attnv(h) [q,d]-oriented + scores(h+1), per q-tile --
        pp_sb = [singles.tile([P, NQ], BF16, name=f"pp{p}", tag=f"pp{p}")
                 for p in range(2)]
        pnap = [singles.tile([P, D + 1], F32, name=f"pna{q}", tag=f"pna{q}")
                for q in range(8)]

        def attnv_ps(h, q, j0, j1):
            ps = ps_sm.tile([P, D + 1], F32, name=f"ps_a{h}_{q}_{j0}", tag="pss")
            for j in range(j0, j1):
                mm(ps, ets[h][j // 2][:, j % 2, q * P:(q + 1) * P],
                   v_sb[j][:, h, :],
                   start=(j == j0), stop=(j == j1 - 1), skip_group_check=True)
            return ps

        def norm_nm(h, q, src):
            rc = rcp.tile([P, 1], F32, name=f"rc{h}_{q}", tag="rc")
            nc.vector.reciprocal(rc, src[:, D:D + 1])
            nm = nump.tile([P, D], BF16, name=f"nm{h}_{q}", tag="nm")
            nc.vector.tensor_scalar_mul(nm, src[:, 0:D], rc)
            return nm

        def attnv(h, q):
            return norm_nm(h, q, attnv_ps(h, q, 0, 16))

        tail_mode = [False]

        def transp(h, q, nm):
            po = D * (h % 2)
            tp = ps_sm.tile([P, P], BF16, name=f"tp{h}_{q}", tag="pss")
            nc.tensor.transpose(tp[po:po + D, :], nm, id_sb)
            evac = nc.scalar.copy if tail_mode[0] else nc.vector.tensor_copy
            evac(pp_sb[h // 2][po:po + D, q * P:(q + 1) * P], tp[po:po + D, :])

        cp_dve = nc.vector.tensor_copy
        cp_act = nc.scalar.copy
        cp_pool = nc.gpsimd.tensor_copy

        def proj(pair, q, out_dram, pool, fin_eng):
            # one head-pair's partial projection; pair 0 completes an
            # exp-window before pair 1, so its 1MB of output DMA streams
            # during the exp(3) window instead of after the last exp.
            # One full-row DMA per q-tile: contiguous 2KB destination rows
            # cost half of two strided half-row transfers, and the queues
            # alternate so neither SP-SEQ nor the Pool-side SWDGE trigger
            # (~1us of Pool engine each) serializes the drain.
            pst = [pool.tile([P, 512], F32, name=f"ps_f{pair}_{q}_{i}",
                             tag="psc" if pool is ps_sc else "pss")
                   for i in range(2)]
            for nh in range(2):
                mm(pst[nh], pp_sb[pair][:, q * P:(q + 1) * P],
                   wp_sb[:, pair, nh * 512:(nh + 1) * 512],
                   start=True, stop=True)
            fin = outp.tile([P, C], BF16, name=f"fin{pair}_{q}", tag="fin")
            if fin_eng[0] is fin_eng[1]:
                # single engine: one wide copy amortizes the access setup
                fin_eng[0](fin[:, 0:512], pst[0])
                fin_eng[0](fin[:, 512:1024], pst[1])
            else:
                fin_eng[0](fin[:, 0:512], pst[0])
                fin_eng[1](fin[:, 512:1024], pst[1])
            dma = nc.sync if q % 2 == 0 else nc.gpsimd
            dma.dma_start(out=out_dram[q * P:(q + 1) * P, :], in_=fin)

        # Emission-window h executes during the exp(h+1) ACT window (the
        # PE FIFO self-paces on the 2-deep scores rotation), so attnv(h')
        # lands two windows after its scores and only attnv(3) + the
        # pair-1 projection follow the last exp:
        #   win0: scores(1) + v(12..15) + k1h0 ci-paced filler + k1h1
        #   win1: scores(2) + attnv(0) [q<4] + attnv(1) [q>=4]
        #   win2: scores(3) + attnv(2) [q<4] + proj_a + out_a DMA
        #   win3: attnv(3) riding the exp(3) tail + proj_b + out_b DMA
        pend = []

        def flush(keep):
            while len(pend) > keep:
                transp(*pend.pop(0))

        k1b_blocks = []

        def k1b_block(half):
            t = ps_sm.tile([P, 512], F32, name=f"ps_k1h1_{half}", tag="pss")
            for ci in range(8):
                mm(t, wk_sb[:, ci, P:2 * P],
                   xt[ci][:, 1024 + half * 512:1024 + (half + 1) * 512],
                   start=(ci == 0), stop=(ci == 7), skip_group_check=True)
            cp_dve(kt[1][:, 1024 + half * 512:1024 + (half + 1) * 512], t)

        for h in range(HPC):
            k1 = None
            if h == 0:
                k1 = [ps_sm.tile([P, 512], F32, name=f"ps_k1h0_{i}", tag="pss")
                      for i in range(2)]
            if h < HPC - 1:
                alloc_ets(h + 1)
            for q in range(8):
                if h < HPC - 1:
                    scores_j(h + 1, 2 * q)
                    scores_j(h + 1, 2 * q + 1)
                if h == 0 and q == 1:
                    # the first exp(1) pair is already in flight; now the
                    # v tail rides win-0's PE slack without delaying it
                    for j in range(11, 16):
                        v_group(j)
                if h == 1 and q == 1:
                    k1b_block(0)
                if h == 1 and q == 2:
                    k1b_block(1)
                if k1 is not None:
                    for half in range(2):
                        mm(k1[half], wk_sb[:, q, P:2 * P],
                           xt[q][:, half * 512:(half + 1) * 512],
                           start=(q == 0), stop=(q == 7),
                           skip_group_check=True)
                # deep lag (flush 3, proj lag 3) keeps every non-scores
                # instruction dep-satisfied when PE reaches it, so only
                # the self-pacing scores pairs occupy the 4-deep wait
                # queue and the filler work runs in the exp-stream slack
                if h == 1:
                    hh, qq0 = (0, 2 * q) if q < 4 else (1, 2 * (q - 4))
                    for qq in (qq0, qq0 + 1):
                        pend.append((hh, qq, attnv(hh, qq)))
                        flush(3)
                elif h == 2:
                    pend.append((2, q, attnv(2, q)))
                    flush(3)
                    if q >= 4:
                        # first-half attnv(3) groups (keys 0..1023): their
                        # exp(3,0..7) deps are satisfied by mid-window, so
                        # they fill win-2's PE slack; parking the partials
                        # in SBUF leaves only the 8-matmul second halves
                        # and their chains after the last exp
                        for qq in (2 * (q - 4), 2 * (q - 4) + 1):
                            cp_dve(pnap[qq], attnv_ps(3, qq, 0, 8))
                    if q >= 1:
                        proj(0, q - 1, out_a, ps_sm, (cp_dve, cp_dve))
            if h == 2:
                proj(0, 7, out_a, ps_sm, (cp_act, cp_dve))
            if k1 is not None:
                # k1h0 evac, then k1h1 reuses the freed slots (its 16
                # matmuls run in window 1's PE slack, before attnv(0))
                for half in range(2):
                    cp_dve(kt[1][:, half * 512:(half + 1) * 512], k1[half])
        # win-3 tail: second-half attnv(3), combine with the parked first
        # halves, transpose, pair-1 projection; fins and transpose evacs
        # ride the now-idle ACT so DVE only carries the combine chain
        tail_mode[0] = True
        flush(0)
        for q in range(8):
            ps = attnv_ps(3, q, 8, 16)
            tmp = nump.tile([P, D + 1], F32, name=f"tmp{q}", tag="tmp", bufs=2)
            nc.vector.tensor_add(tmp, pnap[q], ps)
            pend.append((3, q, norm_nm(3, q, tmp)))
            flush(1)
            if q >= 1:
                proj(1, q - 1, out_b, ps_sc, (cp_act, cp_dve))
        flush(0)
        proj(1, 7, out_b, ps_sc, (cp_act, cp_dve))


def _get_nc():
    if "nc" not in _CACHE:
        _CACHE["nc"] = _build()
    return _CACHE["nc"]


def kernel(x, wq, wk, wv, w_proj, b_proj):
    bf = ml_dtypes.bfloat16
    x = np.asarray(x, dtype=np.float32)
    wq = np.asarray(wq, dtype=np.float32)
    wk = np.asarray(wk, dtype=np.float32)
    wv = np.asarray(wv, dtype=np.float32)
    w_proj = np.asarray(w_proj, dtype=np.float32)
    b_proj = np.asarray(b_proj, dtype=np.float32)
    ident = np.eye(P, dtype=bf)

    nc = _get_nc()
    in_maps = []
    for core in range(8):
        b, g = divmod(core, 4)
        sl = slice(g * DH, (g + 1) * DH)
        in_maps.append({
            "xT": np.ascontiguousarray(x[b].T.astype(bf)),
            "wqT": np.ascontiguousarray(wq[sl, :].T.astype(bf)),
            "wkT": np.ascontiguousarray(wk[sl, :].T.astype(bf)),
            "wvT": np.ascontiguousarray(wv[sl, :].T.astype(bf)),
            "wpT": np.ascontiguousarray(w_proj[:, sl].T.astype(bf)),
            "ident": ident,
        })

    res = run_bass_kernel_spmd(nc, in_maps, core_ids=list(range(8)),
                               trace=bool(int(os.environ.get("KERNEL_TRACE", "0"))))
    _CACHE["last_results"] = res
    outs = [res.results[c]["out_a"].astype(np.float32)
            + res.results[c]["out_b"].astype(np.float32) for c in range(8)]
    full = np.stack([outs[0] + outs[1] + outs[2] + outs[3],
                     outs[4] + outs[5] + outs[6] + outs[7]])
    full += b_proj[None, None, :]
    return full.astype(np.float32)


# revision 47
# speedup vs baseline: 1.0415x; 1.0016x over previous
"""Cross-attention kernel for Trainium2, 8-core SPMD.

Problem (all fp32):
  x [2, 2048, 1024]; wq/wk/wv/w_proj [1024, 1024]; b_proj [1024]
  q = x[:, :1024] @ wq.T   (16 heads x 64)
  k, v = x @ wk.T, x @ wv.T
  out = softmax(q k^T / 8) v  -> proj + bias  -> [2, 1024, 1024]

Sharding: 8 cores = 2 (batch) x 4 (head-groups of 4 heads). Each core
computes its batch's QKV for its 4 heads, full attention for those heads,
and a partial projection (its 256 contraction rows of w_proj). Host sums
the 4 bf16 partials per batch in fp32 and adds the bias.

Numerics: all DRAM inputs are bf16 (halves the input DMA, which bounds
how early the exp stream can start); q/k and the scores matmul stay in
fp32(r); exp(scores), v, attention output and the projection run in bf16
(validated 5.4e-3 max rel err vs the 2e-2 gate).

Layouts: x is kept feature-on-partition (xT [c, n]); q/k are produced
transposed (qT/kT [dh_pair, n]); v natural [n, d] with an appended
ones-column so attn@v also emits the softmax denominator. attn@v is
oriented [q, d] (queries on partitions): the output free size is 65
instead of 512, which halves its PE cost since PE time is
output-free-size * steps. The per-(head, q-tile) [128, 65] PSUM result is
normalized during evacuation (reciprocal of the denominator column +
per-partition tensor_scalar broadcast), PE-transposed into a packed
[2-heads x 64, q] tile via tile_position, and the projection then runs
with full 128-row stationary operands (half the naive cost).

Schedule: the ACT engine's exp stream (~66us: 64 x [128,1024] exps) is
the stage-B pacer; scores(h+1) and attnv(h) interleave per q-tile so ACT
never starves, and the projection pipelines per-q-tile inside head 3's
exp window. Stage A is paced by the 5MB x+wq+wk DMA stream (~19us);
scores(0)/exp(0) start immediately after the last x chunk, with the
remaining q/k/v work used as PE filler between them.
"""

import os
import numpy as np
import ml_dtypes

import concourse.bacc as bacc
import concourse.bass as bass
import concourse.tile as tile
import concourse.mybir as mybir
from concourse.bass_utils import run_bass_kernel_spmd

F32 = mybir.dt.float32
F32R = mybir.dt.float32r
BF16 = mybir.dt.bfloat16

C = 1024          # model dim
N = 2048          # kv tokens
NQ = 1024         # query tokens
HPC = 4           # heads per core
D = 64            # head dim
DH = HPC * D      # per-core slice of C (256)
SCALE = D ** -0.5
P = 128

_CACHE: dict = {}


def _build():
    nc = bacc.Bacc("TRN2", target_bir_lowering=False, debug=False, num_devices=8)

    xT = nc.dram_tensor("xT", [C, N], BF16, kind="ExternalInput").ap()
    wqT = nc.dram_tensor("wqT", [C, DH], BF16, kind="ExternalInput").ap()
    wkT = nc.dram_tensor("wkT", [C, DH], BF16, kind="ExternalInput").ap()
    wvT = nc.dram_tensor("wvT", [C, DH], BF16, kind="ExternalInput").ap()
    wpT = nc.dram_tensor("wpT", [DH, C], BF16, kind="ExternalInput").ap()
    ident = nc.dram_tensor("ident", [P, P], BF16, kind="ExternalInput").ap()
    out_a = nc.dram_tensor("out_a", [NQ, C], BF16, kind="ExternalOutput").ap()
    out_b = nc.dram_tensor("out_b", [NQ, C], BF16, kind="ExternalOutput").ap()

    with tile.TileContext(nc) as tc, \
            nc.allow_low_precision(reason="bf16/fp32r attention pipeline, validated 5.4e-3 rel err"):
        _emit(tc, xT, wqT, wkT, wvT, wpT, ident, out_a, out_b)

    nc.compile()
    return nc


def _emit(tc, xT, wqT, wkT, wvT, wpT, ident, out_a, out_b):
    nc = tc.nc
    mm = nc.tensor.matmul
    Exp = mybir.ActivationFunctionType.Exp

    from contextlib import ExitStack

    with ExitStack() as ctx:
        xp = ctx.enter_context(tc.tile_pool(name="xp", bufs=8))
        wts = ctx.enter_context(tc.tile_pool(name="wts", bufs=1))
        singles = ctx.enter_context(tc.tile_pool(name="singles", bufs=1))
        etsp = ctx.enter_context(tc.tile_pool(name="etsp", bufs=24))
        nump = ctx.enter_context(tc.tile_pool(name="nump", bufs=4))
        rcp = ctx.enter_context(tc.tile_pool(name="rcp", bufs=4))
        outp = ctx.enter_context(tc.tile_pool(name="outp", bufs=6))
        # PSUM: scores triple-buffer (2-deep leaves ~1us/pair ACT idle on
        # the slot-free round-trip) + two 2KB slots = exactly
        # 16KB/partition. During the x DMA stream all 16KB hold the 4
        # ci-paced q/k groups; in window 3 the triple-buffer carries the
        # pair-1 projection instead of scores.
        ps_sc = ctx.enter_context(tc.tile_pool(name="ps_sc", bufs=2, space="PSUM"))
        ps_sm = ctx.enter_context(tc.tile_pool(name="ps_sm", bufs=4, space="PSUM"))

        # ---- input DMAs: per-ci (wq, wk, x) so a1 consumes chunks as they
        # arrive; wv/wp/ident stream after x (not needed until later).
        wq_src = wqT.rearrange("(a p) d -> p a d", p=P)
        wk_src = wkT.rearrange("(a p) d -> p a d", p=P)
        wv_src = wvT.rearrange("(a p) d -> p a d", p=P)
        wq_sb = wts.tile([P, 8, DH], BF16, name="wq_sb", tag="wq")
        wk_sb = wts.tile([P, 8, DH], BF16, name="wk_sb", tag="wk")
        wv_sb = wts.tile([P, 8, DH], BF16, name="wv_sb", tag="wv")
        xt = [xp.tile([P, N], BF16, name=f"xt{ci}", tag="xt") for ci in range(8)]
        for ci in range(8):
            nc.sync.dma_start(out=wq_sb[:, ci, :], in_=wq_src[:, ci, :])
            nc.gpsimd.dma_start(out=wk_sb[:, ci, :], in_=wk_src[:, ci, :])
            eng = nc.sync if ci % 2 == 0 else nc.gpsimd
            eng.dma_start(out=xt[ci], in_=xT[ci * P:(ci + 1) * P, :])
        for ci in range(8):
            eng = nc.sync if ci % 2 == 0 else nc.gpsimd
            eng.dma_start(out=wv_sb[:, ci, :], in_=wv_src[:, ci, :])
        wp_sb = wts.tile([P, 2, C], BF16, name="wp_sb", tag="wp")
        wp_src = wpT.rearrange("(a p) d -> p a d", p=P)
        nc.sync.dma_start(out=wp_sb[:, 0, :], in_=wp_src[:, 0, :])
        nc.gpsimd.dma_start(out=wp_sb[:, 1, :], in_=wp_src[:, 1, :])
        id_sb = singles.tile([P, P], BF16, name="id_sb", tag="id")
        nc.sync.dma_start(out=id_sb, in_=ident)

        # Pre-trigger the ~2.7us exp table load while DMAs stream.
        dm = singles.tile([1, 2], F32, name="dm", tag="dm")
        nc.vector.memset(dm[:, 0:1], 0.0)
        nc.scalar.activation(out=dm[:, 1:2], in_=dm[:, 0:1], func=Exp, scale=1.0)

        # v tiles: [tokens, head, d+1]; col 64 = ones (denominator trick)
        v_sb = [singles.tile([P, HPC, D + 1], BF16, name=f"v{j}", tag=f"v{j}")
                for j in range(16)]
        for j in range(16):
            nc.gpsimd.memset(v_sb[j][:, :, D:D + 1], 1.0)

        qt = [singles.tile([P, NQ], F32R, name=f"qt{p}", tag=f"qt{p}") for p in range(2)]
        kt = [singles.tile([P, N], F32R, name=f"kt{p}", tag=f"kt{p}") for p in range(2)]

        # ---- a1: q(pair0), k(pair0) both halves, and q(pair1) accumulate
        # ci-paced in all 16KB of PSUM while the x chunks stream in. Only
        # k(pair1) (needed first by scores(2), two exp-windows later) is
        # left for the head-0/1 windows.
        ps_q0 = ps_sc.tile([P, NQ], F32, name="ps_q0", tag="psc")
        ps_k0a = ps_sc.tile([P, NQ], F32, name="ps_k0a", tag="psc")
        ps_k0b = [ps_sm.tile([P, 512], F32, name=f"ps_k0b_{i}", tag="pss")
                  for i in range(2)]
        ps_q1 = [ps_sm.tile([P, 512], F32, name=f"ps_q1_{i}", tag="pss")
                 for i in range(2)]

        def a1_ci(ci, which):
            ss = dict(start=(ci == 0), stop=(ci == 7), skip_group_check=True)
            for nh in range(2):
                sl = slice(nh * 512, (nh + 1) * 512)
                if which in ("qk0", "all"):
                    mm(ps_q0[:, sl], wq_sb[:, ci, 0:P], xt[ci][:, sl], **ss)
                    mm(ps_k0a[:, sl], wk_sb[:, ci, 0:P], xt[ci][:, sl], **ss)
                if which in ("rest", "all"):
                    mm(ps_k0b[nh], wk_sb[:, ci, 0:P],
                       xt[ci][:, 1024 + nh * 512:1024 + (nh + 1) * 512], **ss)
                    mm(ps_q1[nh], wq_sb[:, ci, P:2 * P], xt[ci][:, sl], **ss)

        # ci 7 only runs the two groups the first scores need before the
        # evacuations; its other half drains behind the first exps.
        for ci in range(7):
            a1_ci(ci, "all")
        a1_ci(7, "qk0")
        # parallel evac: q0 on DVE, k0 first half on Pool, so scores(0,0)
        # can issue after ~0.6us of copies.
        nc.vector.tensor_copy(qt[0][:, 0:512], ps_q0[:, 0:512])
        nc.scalar.copy(kt[0][:, 0:512], ps_k0a[:, 0:512])
        nc.vector.tensor_copy(qt[0][:, 512:1024], ps_q0[:, 512:1024])
        nc.scalar.copy(kt[0][:, 512:1024], ps_k0a[:, 512:1024])

        # ---- scores/exp helpers ------------------------------------------
        # Three heads of ets tiles are live at once (consume h-1, feed
        # attnv h, write h+1): ets[0]/ets[1]/ets[3] use the 16-slot pool,
        # ets[2] reuses the xt slots (same 4KB; xt's last readers are the
        # k(pair1) matmuls at the end of window 0).
        ets = {}

        def alloc_ets(h):
            pool, tag = (xp, "xt") if h == 2 else (etsp, "ets")
            ets[h] = [pool.tile([P, 2, NQ], BF16, name=f"et{h}_{k}", tag=tag)
                      for k in range(8)]

        def scores_j(h, j):
            pair, po = h // 2, D * (h % 2)
            ps = ps_sc.tile([P, NQ], F32, name=f"ps_s{h}_{j}", tag="psc")
            lw = kt[pair][po:po + D, j * P:(j + 1) * P]
            for nh in range(2):
                mm(ps[:, nh * 512:(nh + 1) * 512], lw,
                   qt[pair][po:po + D, nh * 512:(nh + 1) * 512],
                   start=True, stop=True)
            nc.scalar.activation(out=ets[h][j // 2][:, j % 2, :], in_=ps,
                                 func=Exp, scale=SCALE)

        # v projection: one 8-step psum group per token chunk
        def v_group(j):
            ps = ps_sm.tile([P, DH], F32, name=f"ps_v{j}", tag="pss")
            for ci in range(8):
                mm(ps, xt[ci][:, j * P:(j + 1) * P], wv_sb[:, ci, :],
                   start=(ci == 0), stop=(ci == 7), skip_group_check=True)
            nc.vector.tensor_copy(
                v_sb[j][:, :, 0:D], ps.rearrange("p (h d) -> p h d", h=HPC))

        # ---- B0 prelude: exp(0) paces everything; the v-groups ride
        # along as PE filler and the leftover a1 work drains behind the
        # first two exps.
        alloc_ets(0)
        scores_j(0, 0)
        scores_j(0, 1)
        a1_ci(7, "rest")
        nc.vector.tensor_copy(kt[0][:, 1024:1536], ps_k0b[0])
        nc.vector.tensor_copy(kt[0][:, 1536:2048], ps_k0b[1])
        for nh in range(2):
            nc.vector.tensor_copy(qt[1][:, nh * 512:(nh + 1) * 512], ps_q1[nh])
        for j in range(2, 16):
            if j - 2 < 11:
                v_group(j - 2)
            scores_j(0, j)

        # ---- stage B: attnv(h) [q,d]-oriented + scores(h+1), per q-tile --
        pp_sb = [singles.tile([P, NQ], BF16, name=f"pp{p}", tag=f"pp{p}")
                 for p in range(2)]
        pnap = [singles.tile([P, D + 1], F32, name=f"pna{q}", tag=f"pna{q}")
                for q in range(8)]

        def attnv_ps(h, q, j0, j1):
            ps = ps_sm.tile([P, D + 1], F32, name=f"ps_a{h}_{q}_{j0}", tag="pss")
            for j in range(j0, j1):
                mm(ps, ets[h][j // 2][:, j % 2, q * P:(q + 1) * P],
                   v_sb[j][:, h, :],
                   start=(j == j0), stop=(j == j1 - 1), skip_group_check=True)
            return ps

        def norm_nm(h, q, src):
            rc = rcp.tile([P, 1], F32, name=f"rc{h}_{q}", tag="rc")
            nc.vector.reciprocal(rc, src[:, D:D + 1])
            nm = nump.tile([P, D], BF16, name=f"nm{h}_{q}", tag="nm")
            nc.vector.tensor_scalar_mul(nm, src[:, 0:D], rc)
            return nm

        def attnv(h, q):
            return norm_nm(h, q, attnv_ps(h, q, 0, 16))

        tail_mode = [False]

        def transp(h, q, nm):
            po = D * (h % 2)
            tp = ps_sm.tile([P, P], BF16, name=f"tp{h}_{q}", tag="pss")
            nc.tensor.transpose(tp[po:po + D, :], nm, id_sb)
            evac = nc.scalar.copy if tail_mode[0] else nc.vector.tensor_copy
            evac(pp_sb[h // 2][po:po + D, q * P:(q + 1) * P], tp[po:po + D, :])

        cp_dve = nc.vector.tensor_copy
        cp_act = nc.scalar.copy
        cp_pool = nc.gpsimd.tensor_copy

        def proj(pair, q, out_dram, pool, fin_eng):
            # one head-pair's partial projection; pair 0 completes an
            # exp-window before pair 1, so its 1MB of output DMA streams
            # during the exp(3) window instead of after the last exp.
            # One full-row DMA per q-tile: contiguous 2KB destination rows
            # cost half of two strided half-row transfers, and the queues
            # alternate so neither SP-SEQ nor the Pool-side SWDGE trigger
            # (~1us of Pool engine each) serializes the drain.
            pst = [pool.tile([P, 512], F32, name=f"ps_f{pair}_{q}_{i}",
                             tag="psc" if pool is ps_sc else "pss")
                   for i in range(2)]
            for nh in range(2):
                mm(pst[nh], pp_sb[pair][:, q * P:(q + 1) * P],
                   wp_sb[:, pair, nh * 512:(nh + 1) * 512],
                   start=True, stop=True)
            fin = outp.tile([P, C], BF16, name=f"fin{pair}_{q}", tag="fin")
            if fin_eng[0] is fin_eng[1]:
                # single engine: one wide copy amortizes the access setup
                fin_eng[0](fin[:, 0:512], pst[0])
                fin_eng[0](fin[:, 512:1024], pst[1])
            else:
                fin_eng[0](fin[:, 0:512], pst[0])
                fin_eng[1](fin[:, 512:1024], pst[1])
            dma = nc.sync if q % 2 == 0 else nc.gpsimd
            dma.dma_start(out=out_dram[q * P:(q + 1) * P, :], in_=fin)

        # Emission-window h executes during the exp(h+1) ACT window (the
        # PE FIFO self-paces on the 2-deep scores rotation), so attnv(h')
        # lands two windows after its scores and only attnv(3) + the
        # pair-1 projection follow the last exp:
        #   win0: scores(1) + v(12..15) + k1h0 ci-paced filler + k1h1
        #   win1: scores(2) + attnv(0) [q<4] + attnv(1) [q>=4]
        #   win2: scores(3) + attnv(2) [q<4] + proj_a + out_a DMA
        #   win3: attnv(3) riding the exp(3) tail + proj_b + out_b DMA
        pend = []

        def flush(keep):
            while len(pend) > keep:
                transp(*pend.pop(0))

        k1b_blocks = []

        def k1b_block(half):
            t = ps_sm.tile([P, 512], F32, name=f"ps_k1h1_{half}", tag="pss")
            for ci in range(8):
                mm(t, wk_sb[:, ci, P:2 * P],
                   xt[ci][:, 1024 + half * 512:1024 + (half + 1) * 512],
                   start=(ci == 0), stop=(ci == 7), skip_group_check=True)
            cp_dve(kt[1][:, 1024 + half * 512:1024 + (half + 1) * 512], t)

        for h in range(HPC):
            k1 = None
            if h == 0:
                k1 = [ps_sm.tile([P, 512], F32, name=f"ps_k1h0_{i}", tag="pss")
                      for i in range(2)]
            if h < HPC - 1:
                alloc_ets(h + 1)
            for q in range(8):
                if h < HPC - 1:
                    scores_j(h + 1, 2 * q)
                    scores_j(h + 1, 2 * q + 1)
                if h == 0 and q == 1:
                    # the first exp(1) pair is already in flight; now the
                    # v tail rides win-0's PE slack without delaying it
                    for j in range(11, 16):
                        v_group(j)
                if h == 1 and q == 1:
                    k1b_block(0)
                if h == 1 and q == 2:
                    k1b_block(1)
                if k1 is not None:
                    for half in range(2):
                        mm(k1[half], wk_sb[:, q, P:2 * P],
                           xt[q][:, half * 512:(half + 1) * 512],
                           start=(q == 0), stop=(q == 7),
                           skip_group_check=True)
                # deep lag (flush 3, proj lag 3) keeps every non-scores
                # instruction dep-satisfied when PE reaches it, so only
                # the self-pacing scores pairs occupy the 4-deep wait
                # queue and the filler work runs in the exp-stream slack
                if h == 1:
                    hh, qq0 = (0, 2 * q) if q < 4 else (1, 2 * (q - 4))
                    for qq in (qq0, qq0 + 1):
                        pend.append((hh, qq, attnv(hh, qq)))
                        flush(3)
                elif h == 2:
                    pend.append((2, q, attnv(2, q)))
                    flush(3)
                    if q >= 4:
                        # first-half attnv(3) groups (keys 0..1023): their
                        # exp(3,0..7) deps are satisfied by mid-window, so
                        # they fill win-2's PE slack; parking the partials
                        # in SBUF leaves only the 8-matmul second halves
                        # and their chains after the last exp
                        for qq in (2 * (q - 4), 2 * (q - 4) + 1):
                            cp_dve(pnap[qq], attnv_ps(3, qq, 0, 8))
                    if q >= 1:
                        proj(0, q - 1, out_a, ps_sm, (cp_dve, cp_dve))
            if h == 2:
                proj(0, 7, out_a, ps_sm, (cp_act, cp_dve))
            if k1 is not None:
                # k1h0 evac, then k1h1 reuses the freed slots (its 16
                # matmuls run in window 1's PE slack, before attnv(0))
                for half in range(2):
                    cp_dve(kt[1][:, half * 512:(half + 1) * 512], k1[half])
        # win-3 tail: second-half attnv(3), combine with the parked first
        # halves, transpose, pair-1 projection; fins and transpose evacs
        # ride the now-idle ACT so DVE only carries the combine chain
        tail_mode[0] = True
        flush(0)
        for q in range(8):
            ps = attnv_ps(3, q, 8, 16)
            tmp = nump.tile([P, D + 1], F32, name=f"tmp{q}", tag="tmp", bufs=2)
            nc.vector.tensor_add(tmp, pnap[q], ps)
            pend.append((3, q, norm_nm(3, q, tmp)))
            flush(1)
            if q >= 1:
                proj(1, q - 1, out_b, ps_sc, (cp_act, cp_dve))
        flush(0)
        proj(1, 7, out_b, ps_sc, (cp_act, cp_dve))


def _get_nc():
    if "nc" not in _CACHE:
        _CACHE["nc"] = _build()
    return _CACHE["nc"]


def kernel(x, wq, wk, wv, w_proj, b_proj):
    bf = ml_dtypes.bfloat16
    x = np.asarray(x, dtype=np.float32)
    wq = np.asarray(wq, dtype=np.float32)
    wk = np.asarray(wk, dtype=np.float32)
    wv = np.asarray(wv, dtype=np.float32)
    w_proj = np.asarray(w_proj, dtype=np.float32)
    b_proj = np.asarray(b_proj, dtype=np.float32)
    ident = np.eye(P, dtype=bf)

    nc = _get_nc()
    in_maps = []
    for core in range(8):
        b, g = divmod(core, 4)
        sl = slice(g * DH, (g + 1) * DH)
        in_maps.append({
            "xT": np.ascontiguousarray(x[b].T.astype(bf)),
            "wqT": np.ascontiguousarray(wq[sl, :].T.astype(bf)),
            "wkT": np.ascontiguousarray(wk[sl, :].T.astype(bf)),
            "wvT": np.ascontiguousarray(wv[sl, :].T.astype(bf)),
            "wpT": np.ascontiguousarray(w_proj[:, sl].T.astype(bf)),
            "ident": ident,
        })

    res = run_bass_kernel_spmd(nc, in_maps, core_ids=list(range(8)),
                               trace=bool(int(os.environ.get("KERNEL_TRACE", "0"))))
    _CACHE["last_results"] = res
    outs = [res.results[c]["out_a"].astype(np.float32)
            + res.results[c]["out_b"].astype(np.float32) for c in range(8)]
    full = np.stack([outs[0] + outs[1] + outs[2] + outs[3],
                     outs[4] + outs[5] + outs[6] + outs[7]])
    full += b_proj[None, None, :]
    return full.astype(np.float32)
